# revision 12
# baseline (speedup 1.0000x reference)
"""Trainium2 Bass kernel for nn_MultiHeadAttention_42185168781349.

Quirky MHA layout: reshape(b,s,h,d).transpose(0,3,1,2) makes d_model=64 act
as 64 attention heads with head_dim=64 (head index = col % 64 of the 4096
projection columns). B=4, S=1024.

Sharding: 8 cores = 4 batches x 2 head-groups of 32 heads (tensor-parallel
split of the projection columns / dense rows). Each core computes a partial
(64, 1024) output^T for its batch; host sums the two head-group partials and
transposes.

Per-core device pipeline (flash-style, scores never leave SBUF):
  phase 0: Q^T/K^T pair projections (65-row augmented matmul folds bias),
           V in (t, h) orientation with a ones column appended (row-sum trick)
  per head: scores^T = K^T-chunk.T @ Q^T  (psum, fp32)
            P = exp(scores^T)             (ScalarE, -> bf16 sbuf)
            attout^T/sumexp = [V|1].T @ P (psum accumulate over t-chunks)
            recip(sumexp) -> partition_broadcast -> normalize (DVE/GpSimd)
            out_acc += Wd_head.T @ attout^T  (psum -> DVE accumulate)
No max-subtraction needed: |scores| <= ~8 so exp is safe in fp32.
"""

import sys

sys.path.insert(0, "/opt/trn_rl_repo")

from contextlib import ExitStack

import numpy as np
import ml_dtypes

import concourse.bass as bass
import concourse.tile as tile
from concourse import mybir
from concourse.bass_utils import run_bass_kernel_spmd
from concourse.vector_clock import ScopedClock, VectorClock
from concourse.tile_scheduler import N_PROCS

BF = mybir.dt.bfloat16
F32 = mybir.dt.float32
BF_NP = ml_dtypes.bfloat16

B, S, D = 4, 1024, 64  # batch, seq, d_model (= num heads here)
H = 64                 # per-head feature dim
HPC = 32               # heads per core
N_CORES = 8


# ---------------------------------------------------------------------------
# The stock kernel-tail Drain carries one sem-wait per outstanding proc;
# walrus CoreV3 rejects >1 sync wait on a TPB_CTRL instruction. Split the
# waits across single-wait nofuse NOPs ahead of a wait-free drain.
def _patched_drain_and_barrier(self, tick_clock, wait_clock):
    gc = tick_clock.global_clock
    ticks = [gc[p] for p in range(N_PROCS)]
    for p in range(N_PROCS):
        if ticks[p] > 0:
            part = VectorClock([ticks[q] if q == p else 0 for q in range(N_PROCS)])
            nop = self.nc.sync.nop(nofuse=True)
            wait_clock.add_sem_waits(nop.ins, ScopedClock({None: part}))
    self.nc.sync.drain()
    self.nc.all_engine_barrier()
    assert self.sems is not None
    popped = self.nc._tile_sem_poison_stack.pop()
    assert popped is self._sem_poison
    self.nc.clear_and_free_semaphores(list(self.sems.allocated().values()))
    self.nc.all_engine_barrier()


tile.TileContext._drain_and_barrier = _patched_drain_and_barrier

# Walrus CoreV2/V3 codegen also accepts at most ONE sync wait on TPB compute
# instructions (matmul/activation/DVE/...). Tile's wait-assignment pass can
# attach 2+. Before lowering, peel extra waits off onto same-engine NoOps
# inserted immediately ahead of the instruction (engine stalls at the same
# point either way).
_orig_lower_ordered = tile.TileContext._lower_ordered_insts
_WSPLIT = [0]
_WAIT_EXEMPT = {"EventSemaphore", "TriggeredCopy"}


def _lower_with_wait_split(self, ordered):
    for bb_name, insts in ordered.items():
        out = []
        for inst in insts:
            si = inst.sync_info
            op = str(inst.opcode)
            # gpsimd custom ISA instructions cannot carry sync waits at all
            keep = 0 if op == "ISA" else 1
            if (si is not None and si.on_wait and len(si.on_wait) > keep
                    and op not in _WAIT_EXEMPT):
                waits = list(si.on_wait)
                for w in waits[: len(waits) - keep]:
                    _WSPLIT[0] += 1
                    nop = mybir.InstNoOp(name=f"WSPLIT-{_WSPLIT[0]}")
                    nop.engine = inst.engine
                    nop.sync_info = mybir.SyncInfo(on_wait=[w], on_update=[])
                    nop.bass_scheduled_tick = inst.bass_scheduled_tick
                    nop.bass_scheduled_proc = inst.bass_scheduled_proc
                    out.append(nop)
                si.on_wait = waits[len(waits) - keep:]
            out.append(inst)
        ordered[bb_name] = out
    return _orig_lower_ordered(self, ordered)


tile.TileContext._lower_ordered_insts = _lower_with_wait_split
# ---------------------------------------------------------------------------


def _build_nc() -> bass.Bass:
    nc = bass.Bass("TRN2", target_bir_lowering=False, debug=False,
                   num_devices=N_CORES)
    xt_d = nc.dram_tensor("xt", [65, S], BF, kind="ExternalInput").ap()
    wq_d = nc.dram_tensor("wq", [65, HPC * H], BF, kind="ExternalInput").ap()
    wk_d = nc.dram_tensor("wk", [65, HPC * H], BF, kind="ExternalInput").ap()
    wv_d = nc.dram_tensor("wv", [65, HPC * H], BF, kind="ExternalInput").ap()
    wd_d = nc.dram_tensor("wd", [64, HPC * D], BF, kind="ExternalInput").ap()
    bd_d = nc.dram_tensor("bdc", [64, 1], F32, kind="ExternalInput").ap()
    out_d = nc.dram_tensor("out", [64, S], F32, kind="ExternalOutput").ap()

    EXP = mybir.ActivationFunctionType.Exp

    with tile.TileContext(nc) as tc, ExitStack() as ctx:
        singles = ctx.enter_context(tc.tile_pool(name="singles", bufs=1))
        pP = ctx.enter_context(tc.tile_pool(name="pP", bufs=2))
        pAo = ctx.enter_context(tc.tile_pool(name="pAo", bufs=2))
        pBc = ctx.enter_context(tc.tile_pool(name="pBc", bufs=2))
        pRc = ctx.enter_context(tc.tile_pool(name="pRc", bufs=2))
        pDr = ctx.enter_context(tc.tile_pool(name="pDr", bufs=2, space="DRAM"))
        psA = ctx.enter_context(tc.tile_pool(name="psA", bufs=2, space="PSUM"))
        psB = ctx.enter_context(tc.tile_pool(name="psB", bufs=1, space="PSUM"))
        psC = ctx.enter_context(tc.tile_pool(name="psC", bufs=2, space="PSUM"))

        xt = singles.tile([65, S], BF, tag="xt")
        nc.sync.dma_start(out=xt, in_=xt_d)
        wq = singles.tile([65, HPC * H], BF, tag="wq")
        nc.sync.dma_start(out=wq, in_=wq_d)
        wk = singles.tile([65, HPC * H], BF, tag="wk")
        nc.sync.dma_start(out=wk, in_=wk_d)
        wv = singles.tile([65, HPC * H], BF, tag="wv")
        nc.sync.dma_start(out=wv, in_=wv_d)
        wd = singles.tile([64, HPC * D], BF, tag="wd")
        nc.sync.dma_start(out=wd, in_=wd_d)
        bd_sb = singles.tile([64, 1], F32, tag="bd")
        nc.sync.dma_start(out=bd_sb, in_=bd_d)
        out_acc = singles.tile([64, S], F32, tag="oacc")

        # ---- phase 0: projections --------------------------------------
        qTs, kTs = [], []
        for p in range(HPC // 2):
            qT = singles.tile([128, S], BF, tag=f"qT{p}")
            kT = singles.tile([128, S], BF, tag=f"kT{p}")
            for wt, dst in ((wq, qT), (wk, kT)):
                for h in range(2):
                    t = psC.tile([128, 512], F32, tag="c")
                    nc.tensor.matmul(
                        t, wt[:, 128 * p : 128 * (p + 1)],
                        xt[:, 512 * h : 512 * (h + 1)], start=True, stop=True)
                    nc.vector.tensor_copy(dst[:, 512 * h : 512 * (h + 1)], t)
            qTs.append(qT)
            kTs.append(kT)

        vggs = []
        for g in range(HPC // 4):
            # (128 t, 8 chunks, 4 heads x [64 V cols + 1 ones col])
            vg = singles.tile([128, 8, 4 * 65], BF, tag=f"vg{g}")
            nc.vector.memset(
                vg.rearrange("p c (j q) -> p c j q", j=4)[:, :, :, 64:65], 1.0)
            for c in range(8):
                t = psC.tile([128, 512], F32, tag="c")
                nc.tensor.matmul(
                    t[:, 0:256], xt[:, 128 * c : 128 * (c + 1)],
                    wv[:, 256 * g : 256 * (g + 1)], start=True, stop=True)
                src = t[:, 0:256].rearrange("p (j q) -> p j q", j=4)
                dst = vg[:, c, :].rearrange("p (j q) -> p j q", j=4)[:, :, 0:64]
                nc.vector.tensor_copy(dst, src)
            vggs.append(vg)

        # ---- main loop over heads --------------------------------------
        for i in range(HPC):
            pr, j, g, jj = i // 2, i % 2, i // 4, i % 4
            qs = qTs[pr][64 * j : 64 * (j + 1), :]
            ks = kTs[pr][64 * j : 64 * (j + 1), :]
            vg = vggs[g]

            P = pP.tile([128, 8, S], BF, tag="P")
            for c in range(8):
                sc = psA.tile([128, S], F32, tag="sc")
                for h in range(2):
                    nc.tensor.matmul(
                        sc[:, 512 * h : 512 * (h + 1)],
                        ks[:, 128 * c : 128 * (c + 1)],
                        qs[:, 512 * h : 512 * (h + 1)], start=True, stop=True)
                nc.scalar.activation(P[:, c, :], sc, EXP)

            po = psB.tile([128, S], F32, tag="pv")
            for c in range(8):
                for h in range(2):
                    nc.tensor.matmul(
                        po[0:65, 512 * h : 512 * (h + 1)],
                        vg[:, c, 65 * jj : 65 * jj + 65],
                        P[:, c, 512 * h : 512 * (h + 1)],
                        start=(c == 0), stop=(c == 7))

            rc = pRc.tile([1, S], F32, tag="rc")
            nc.vector.reciprocal(rc, po[64:65, :])
            dr = pDr.tile([1, S], F32, tag="dr")
            nc.sync.dma_start(out=dr, in_=rc)
            bc = pBc.tile([64, S], F32, tag="bc")
            nc.sync.dma_start(out=bc, in_=dr.to_broadcast([64, S]))
            ao = pAo.tile([64, S], BF, tag="ao")
            nc.vector.tensor_mul(ao, po[0:64, :], bc)

            for h in range(2):
                t = psC.tile([128, 512], F32, tag="c")
                nc.tensor.matmul(
                    t[0:64, :], wd[:, 64 * i : 64 * (i + 1)],
                    ao[:, 512 * h : 512 * (h + 1)], start=True, stop=True)
                dst = out_acc[:, 512 * h : 512 * (h + 1)]
                if i == 0:
                    nc.vector.tensor_scalar_add(dst, t[0:64, :], bd_sb)
                else:
                    nc.vector.tensor_add(dst, dst, t[0:64, :])

        nc.sync.dma_start(out=out_d, in_=out_acc)
    return nc


def _prep_inputs(x, mask, Wq, bq, Wk, bk, Wv, bv, Wd, bd):
    """Build the 8 per-core input maps (host-side sharding/layout only)."""
    x = np.asarray(x, np.float32)
    Wq_r = (np.asarray(Wq, np.float32) / 8.0).reshape(D, H, D)   # (c, h, d)
    bq_r = (np.asarray(bq, np.float32) / 8.0).reshape(H, D)      # (h, d)
    Wk_r = np.asarray(Wk, np.float32).reshape(D, H, D)
    bk_r = np.asarray(bk, np.float32).reshape(H, D)
    Wv_r = np.asarray(Wv, np.float32).reshape(D, H, D)
    bv_r = np.asarray(bv, np.float32).reshape(H, D)
    # output merge is transpose(0,2,1,3).reshape -> row index = d*H + h
    Wd_r = np.asarray(Wd, np.float32).reshape(D, H, D)           # (d, h, dm)
    bd = np.asarray(bd, np.float32)

    in_maps = []
    for core in range(N_CORES):
        b, dg = core // 2, core % 2
        heads = [dg * HPC + i for i in range(HPC)]

        xt = np.empty((65, S), np.float32)
        xt[0:64] = x[b].T
        xt[64] = 1.0

        def qk_block(w_r, b_r):
            m = np.empty((65, HPC * H), np.float32)
            for i, d in enumerate(heads):
                m[0:64, 64 * i : 64 * (i + 1)] = w_r[:, :, d]
                m[64, 64 * i : 64 * (i + 1)] = b_r[:, d]
            return m

        wqm = qk_block(Wq_r, bq_r)
        wkm = qk_block(Wk_r, bk_r)
        wvm = qk_block(Wv_r, bv_r)  # rhs layout == lhsT layout here

        wdm = np.empty((64, HPC * D), np.float32)
        for i, d in enumerate(heads):
            wdm[:, 64 * i : 64 * (i + 1)] = Wd_r[d, :, :]

        bdc = (bd if dg == 0 else np.zeros_like(bd)).reshape(64, 1)

        in_maps.append({
            "xt": xt.astype(BF_NP),
            "wq": wqm.astype(BF_NP),
            "wk": wkm.astype(BF_NP),
            "wv": wvm.astype(BF_NP),
            "wd": wdm.astype(BF_NP),
            "bdc": bdc.astype(np.float32),
        })
    return in_maps


def _numpy_fallback(x, mask, Wq, bq, Wk, bk, Wv, bv, Wd, bd):
    """Exact fp32 host implementation (only used if mask has zeros)."""
    x = np.asarray(x, np.float32)
    b, s, d = x.shape
    h = H

    def split_heads(y):
        return y.reshape(b, s, h, d).transpose(0, 3, 1, 2)

    q = split_heads(x @ Wq + bq)
    k = split_heads(x @ Wk + bk)
    v = split_heads(x @ Wv + bv)
    scores = np.einsum("bdsh,bdth->bdst", q, k) / np.sqrt(np.float32(d))
    scores = np.where(np.asarray(mask) == 0, np.float32(-1e9), scores)
    scores -= scores.max(axis=-1, keepdims=True)
    e = np.exp(scores)
    attn = e / e.sum(axis=-1, keepdims=True)
    out = np.einsum("bdst,bdth->bdsh", attn, v)
    out = out.transpose(0, 2, 1, 3).reshape(b, s, d * h)
    return (out @ Wd + bd).astype(np.float32)


_NC = None


def _get_nc():
    global _NC
    if _NC is None:
        _NC = _build_nc()
    return _NC


def kernel(x, mask, Wq, bq, Wk, bk, Wv, bv, Wd, bd):
    if not np.all(np.asarray(mask) != 0):
        return _numpy_fallback(x, mask, Wq, bq, Wk, bk, Wv, bv, Wd, bd)

    in_maps = _prep_inputs(x, mask, Wq, bq, Wk, bk, Wv, bv, Wd, bd)
    res = run_bass_kernel_spmd(_get_nc(), in_maps,
                               core_ids=list(range(N_CORES))).results
    out = np.empty((B, S, D), np.float32)
    for b in range(B):
        part = res[2 * b]["out"].astype(np.float32) + res[2 * b + 1]["out"]
        out[b] = part.T
    return out
